# revision 1
# baseline (speedup 1.0000x reference)
"""BiLSTM tagger Trainium kernel — 8-core SPMD, data-parallel over (direction, batch-quarter).

Core i: direction d=i//4 (0=fwd, 1=bwd), batch quarter q=i%4 (rows 16q..16q+16).
All cores run an IDENTICAL program; per-core specialization enters via input data
(bwd cores get time-reversed token indices + their direction's weights).

Per-core pipeline:
  1. embedding gather (indirect DMA) + PE transpose -> X^T tiles (f32r)
  2. z1 = X @ Wih_d1^T + biases  (PE, f32r)  -> DRAM bf16, gate-reordered cols
  3. L1 recurrence, 256 steps:  gates psum [128,512] stacked 4 col-groups
     (col-group j = h-slice j, rows 32j..32j+16 = batch), bf16 matmuls,
     fp32 cell on ACT/DVE, single [128,128] PE transpose h -> h^T.
     h^T history -> SBUF (own order) + DRAM (reversed order, for AllGather).
  4. AllGather(reversed histories) over all 8 cores; indirect-gather the
     peer direction's history (host-provided row indices).
  5. z2 projection (own + peer halves, bf16)  6. L2 recurrence (same as 3)
  7. AllGather + peer gather again  8. FC -> logits [T*16, 50] f32.

Host assembles: cores 0..3 give forward-time logits for quarters 0..3.

Gate-column reorder used everywhere: col(512j + 128g + q) <- gate g (i,f,g,o),
h-slice j, within-slice q.  This puts all 4 gates of one h-slice in one
512-wide psum chunk (one col-group) so the cell runs on [128, *] tiles.
"""
import numpy as np
import ml_dtypes

import concourse.bacc as bacc
import concourse.bass as bass
import concourse.mybir as mybir
import concourse.tile as tile
from concourse.bass_utils import run_bass_kernel_spmd

F32 = mybir.dt.float32
F32R = mybir.dt.float32r
BF16 = mybir.dt.bfloat16
I32 = mybir.dt.int32
AF = mybir.ActivationFunctionType
BF16_NP = ml_dtypes.bfloat16

B, V, E, H, TAGS = 64, 50000, 512, 512, 50
NCORES = 8
BQ = B // 4  # 16: batch rows per core


def _build(T, passes=1):
    """Build + compile the single-core program (identical across cores)."""
    G = T * BQ // 128  # embedding / projection row-groups of 128
    nc = bacc.Bacc("TRN2", target_bir_lowering=False, debug=False,
                   num_devices=NCORES)

    # ---- external inputs (per-core data) ----
    emb_d = nc.dram_tensor("emb", [V, E], F32, kind="ExternalInput").ap()
    idx_d = nc.dram_tensor("idx", [128, G], I32, kind="ExternalInput").ap()
    idxh_d = nc.dram_tensor("idxh", [128, 4], I32, kind="ExternalInput").ap()
    w1_d = nc.dram_tensor("w1", [4, 128, 2048], F32R, kind="ExternalInput").ap()
    b1_d = nc.dram_tensor("b1", [1, 2048], F32R, kind="ExternalInput").ap()
    wm1_d = nc.dram_tensor("wm1", [128, 8192], BF16, kind="ExternalInput").ap()
    w2_d = nc.dram_tensor("w2", [8, 128, 2048], BF16, kind="ExternalInput").ap()
    b2_d = nc.dram_tensor("b2", [1, 2048], BF16, kind="ExternalInput").ap()
    wm2_d = nc.dram_tensor("wm2", [128, 8192], BF16, kind="ExternalInput").ap()
    fcw_d = nc.dram_tensor("fcw", [8, 128, 64], BF16, kind="ExternalInput").ap()
    fcb_d = nc.dram_tensor("fcb", [1, 64], BF16, kind="ExternalInput").ap()
    i16_d = nc.dram_tensor("i16", [16, 32], BF16, kind="ExternalInput").ap()
    onesr_d = nc.dram_tensor("onesr", [1, 128], F32R, kind="ExternalInput").ap()
    onesb_d = nc.dram_tensor("onesb", [1, 128], BF16, kind="ExternalInput").ap()
    id128_d = nc.dram_tensor("id128", [128, 128], F32, kind="ExternalInput").ap()
    logits_d = nc.dram_tensor("logits", [T * BQ, 64], F32,
                              kind="ExternalOutput").ap()

    with tile.TileContext(nc) as tc:
        with tc.tile_pool(name="pconst", bufs=1) as pconst, \
             tc.tile_pool(name="pdram", bufs=1, space="DRAM") as pdram:
            i16 = pconst.tile([16, 32], BF16, name="i16")
            onesr = pconst.tile([1, 128], F32R, name="onesr")
            onesb = pconst.tile([1, 128], BF16, name="onesb")
            id128 = pconst.tile([128, 128], F32, name="id128")
            idxs = pconst.tile([128, G], I32, name="idxs")
            idxh = pconst.tile([128, 4], I32, name="idxh")
            b1r = pconst.tile([1, 2048], F32R, name="b1r")
            b2r = pconst.tile([1, 2048], BF16, name="b2r")
            fcb = pconst.tile([1, 64], BF16, name="fcb")
            nc.sync.dma_start(i16[:], i16_d[:])
            nc.sync.dma_start(onesr[:], onesr_d[:])
            nc.sync.dma_start(onesb[:], onesb_d[:])
            nc.sync.dma_start(id128[:], id128_d[:])
            nc.sync.dma_start(idxs[:], idx_d[:])
            nc.sync.dma_start(idxh[:], idxh_d[:])
            nc.sync.dma_start(b1r[:], b1_d[:])
            nc.sync.dma_start(b2r[:], b2_d[:])
            nc.sync.dma_start(fcb[:], fcb_d[:])

            # internal DRAM
            z1_dram = pdram.tile([T * BQ, 2048], BF16, name="z1_dram")
            z2_dram = pdram.tile([T * BQ, 2048], BF16, name="z2_dram")
            rev1 = pdram.tile([4, 128, T * BQ], BF16, name="rev1")
            rev2 = pdram.tile([4, 128, T * BQ], BF16, name="rev2")
            ag1 = pdram.tile([NCORES * 512, T * BQ], BF16, name="ag1",
                             addr_space="Shared")
            ag2 = pdram.tile([NCORES * 512, T * BQ], BF16, name="ag2",
                             addr_space="Shared")

            for _pass in range(passes):
                # ---------- phase A: embed + z1 projection ----------
                with tc.tile_pool(name="pA", bufs=1) as pA, \
                     tc.tile_pool(name="pAw", bufs=1) as pAw, \
                     tc.tile_pool(name="psA", bufs=4, space="PSUM") as psA, \
                     tc.tile_pool(name="psAt", bufs=2, space="PSUM") as psAt:
                    w1sb = [pAw.tile([128, 2048], F32R, name=f"w1sb{k}")
                            for k in range(4)]
                    for k in range(4):
                        nc.sync.dma_start(w1sb[k][:], w1_d[k])
                    for g in range(G):
                        es = pA.tile([128, 512], F32, tag="es", bufs=3)
                        nc.gpsimd.indirect_dma_start(
                            out=es[:], out_offset=None, in_=emb_d[:],
                            in_offset=bass.IndirectOffsetOnAxis(
                                ap=idxs[:, g:g + 1], axis=0))
                        ptA = psAt.tile([128, 256], F32, tag="ptA")
                        ptB = psAt.tile([128, 256], F32, tag="ptB")
                        for k in range(4):
                            dst = (ptA, ptB)[k // 2]
                            nc.tensor.transpose(
                                dst[:, 128 * (k % 2):128 * (k % 2) + 128],
                                es[:, 128 * k:128 * (k + 1)], id128[:])
                        xt = pA.tile([128, 512], F32R, tag="xt", bufs=3)
                        nc.vector.tensor_copy(xt[:, 0:256], ptA[:])
                        nc.vector.tensor_copy(xt[:, 256:512], ptB[:])
                        zst = pA.tile([128, 2048], BF16, tag="zst", bufs=3)
                        for n in range(4):
                            pg = psA.tile([128, 512], F32, tag="pgA")
                            nc.tensor.matmul(pg[:], lhsT=onesr[:],
                                             rhs=b1r[:, 512 * n:512 * (n + 1)],
                                             start=True, stop=False)
                            for k in range(4):
                                nc.tensor.matmul(
                                    pg[:], lhsT=xt[:, 128 * k:128 * (k + 1)],
                                    rhs=w1sb[k][:, 512 * n:512 * (n + 1)],
                                    start=False, stop=(k == 3))
                            nc.vector.tensor_copy(zst[:, 512 * n:512 * (n + 1)], pg[:])
                        nc.sync.dma_start(z1_dram[128 * g:128 * (g + 1), :], zst[:])

                # ---------- recurrence: two interleaved batch-half chains ----------
                def recurrence(z_dram, wm_sb, hist, rev_dram):
                    CH, BC = 2, 8  # chains per core, batch rows per chain
                    with tc.tile_pool(name="pR", bufs=1) as pR, \
                         tc.tile_pool(name="psR", bufs=1, space="PSUM") as psR, \
                         tc.tile_pool(name="psRt", bufs=1, space="PSUM") as psRt:
                        c_prev, hT_prev = [], []
                        for ch in range(CH):
                            c0 = pR.tile([128, 128], F32, tag=f"c{ch}", bufs=2)
                            hT0 = pR.tile([128, 128], BF16, tag=f"hT{ch}", bufs=2)
                            nc.vector.memset(c0[:], 0.0)
                            nc.vector.memset(hT0[:], 0.0)
                            c_prev.append(c0)
                            hT_prev.append(hT0)
                        for t in range(T):
                            zs = pR.tile([BC, CH * 2048], BF16, tag="z", bufs=8)
                            nc.sync.dma_start(
                                zs[:].rearrange("b (ch c) -> b ch c", ch=CH),
                                z_dram[BQ * t:BQ * (t + 1), :]
                                .rearrange("(ch b) c -> b ch c", ch=CH))
                            for ch in range(CH):
                                zrow = BC * ch
                                pg = psR.tile([128, 512], F32, tag=f"pgR{ch}", bufs=2)
                                for j in range(4):
                                    nc.tensor.matmul(
                                        pg[32 * j:32 * j + 32, :],
                                        lhsT=i16[0:BC, :],
                                        rhs=zs[:, 2048 * ch + 512 * j:2048 * ch + 512 * (j + 1)],
                                        start=True, stop=False,
                                        tile_position=(0, 32 * j),
                                        skip_group_check=True)
                                for k in range(4):
                                    for j in range(4):
                                        nc.tensor.matmul(
                                            pg[32 * j:32 * j + BC, :],
                                            lhsT=hT_prev[ch][:, 32 * k:32 * k + BC],
                                            rhs=wm_sb[:, (k * 4 + j) * 512:(k * 4 + j + 1) * 512],
                                            start=False, stop=(k == 3),
                                            tile_position=(0, 32 * j),
                                            skip_group_check=True)
                                # gates cols: [i|f|o|g] per 512 chunk
                                s_ifo = pR.tile([128, 384], F32, tag=f"sifo{ch}", bufs=2)
                                t_g = pR.tile([128, 128], F32, tag=f"tg{ch}", bufs=2)
                                nc.scalar.activation(s_ifo[:], pg[:, 0:384], AF.Sigmoid)
                                nc.scalar.activation(t_g[:], pg[:, 384:512], AF.Tanh)
                                tmp1 = pR.tile([128, 128], F32, tag=f"tmp1{ch}", bufs=2)
                                tmp2 = pR.tile([128, 128], F32, tag=f"tmp2{ch}", bufs=2)
                                c_new = pR.tile([128, 128], F32, tag=f"c{ch}", bufs=2)
                                nc.vector.tensor_mul(tmp1[:], s_ifo[:, 128:256], c_prev[ch][:])
                                nc.vector.tensor_mul(tmp2[:], s_ifo[:, 0:128], t_g[:])
                                nc.vector.tensor_add(c_new[:], tmp1[:], tmp2[:])
                                t_c = pR.tile([128, 128], F32, tag=f"tc{ch}", bufs=2)
                                nc.scalar.activation(t_c[:], c_new[:], AF.Tanh)
                                h_sb = pR.tile([128, 128], F32, tag=f"h{ch}", bufs=2)
                                nc.vector.tensor_mul(h_sb[:], s_ifo[:, 256:384], t_c[:])
                                pt = psRt.tile([128, 128], F32, tag=f"ptR{ch}", bufs=1)
                                nc.tensor.transpose(pt[:], h_sb[:], id128[:])
                                hT_new = pR.tile([128, 128], BF16, tag=f"hT{ch}", bufs=2)
                                nc.vector.tensor_copy(hT_new[:], pt[:])
                                # own-order history (SBUF) + reversed history (DRAM)
                                for k in range(4):
                                    nc.sync.dma_start(
                                        hist[k][:, BQ * t + zrow:BQ * t + zrow + BC],
                                        hT_new[:, 32 * k:32 * k + BC])
                                tr = T - 1 - t
                                nc.sync.dma_start(
                                    rev_dram[:, :, BQ * tr + zrow:BQ * tr + zrow + BC]
                                    .rearrange("k p b -> p k b"),
                                    hT_new[:].rearrange("p (k b) -> p k b", b=32)[:, :, 0:BC])
                                c_prev[ch], hT_prev[ch] = c_new, hT_new

                # ---------- phase B: L1 recurrence ----------
                with tc.tile_pool(name="pH1", bufs=1) as pH1:
                    hist1 = [pH1.tile([128, T * BQ], BF16, name=f"hist1_{k}")
                             for k in range(4)]
                    with tc.tile_pool(name="pB", bufs=1) as pB:
                        wm1sb = pB.tile([128, 8192], BF16, name="wm1sb")
                        nc.sync.dma_start(wm1sb[:], wm1_d[:])
                        recurrence(z1_dram, wm1sb, hist1, rev1)

                    # ---------- phase C: AG1 + peer gather + z2 ----------
                    nc.gpsimd.collective_compute(
                        "AllGather", mybir.AluOpType.bypass,
                        replica_groups=[list(range(NCORES))],
                        ins=[rev1[:].rearrange("k p t -> (k p) t")],
                        outs=[ag1[:]])
                    with tc.tile_pool(name="pC", bufs=1) as pC, \
                         tc.tile_pool(name="psC", bufs=4, space="PSUM") as psC:
                        xp = [pC.tile([128, T * BQ], BF16, name=f"xp_{k}")
                              for k in range(4)]
                        for k in range(4):
                            nc.gpsimd.indirect_dma_start(
                                out=xp[k][:], out_offset=None, in_=ag1[:],
                                in_offset=bass.IndirectOffsetOnAxis(
                                    ap=idxh[:, k:k + 1], axis=0))
                        w2sb = [pC.tile([128, 2048], BF16, name=f"w2sb{k}")
                                for k in range(8)]
                        for k in range(8):
                            nc.sync.dma_start(w2sb[k][:], w2_d[k])
                        for m in range(G):
                            zst2 = pC.tile([128, 2048], BF16, tag="zst2", bufs=3)
                            for n in range(4):
                                pg = psC.tile([128, 512], F32, tag="pgC")
                                nc.tensor.matmul(pg[:], lhsT=onesb[:],
                                                 rhs=b2r[:, 512 * n:512 * (n + 1)],
                                                 start=True, stop=False)
                                for k in range(8):
                                    st = (hist1[k] if k < 4 else xp[k - 4])
                                    nc.tensor.matmul(
                                        pg[:], lhsT=st[:, 128 * m:128 * (m + 1)],
                                        rhs=w2sb[k][:, 512 * n:512 * (n + 1)],
                                        start=False, stop=(k == 7))
                                nc.vector.tensor_copy(
                                    zst2[:, 512 * n:512 * (n + 1)], pg[:])
                            nc.sync.dma_start(z2_dram[128 * m:128 * (m + 1), :], zst2[:])

                # ---------- phase D: L2 recurrence ----------
                with tc.tile_pool(name="pH2", bufs=1) as pH2:
                    hist2 = [pH2.tile([128, T * BQ], BF16, name=f"hist2_{k}")
                             for k in range(4)]
                    with tc.tile_pool(name="pD", bufs=1) as pD:
                        wm2sb = pD.tile([128, 8192], BF16, name="wm2sb")
                        nc.sync.dma_start(wm2sb[:], wm2_d[:])
                        recurrence(z2_dram, wm2sb, hist2, rev2)

                    # ---------- phase E: AG2 + peer gather + FC ----------
                    nc.gpsimd.collective_compute(
                        "AllGather", mybir.AluOpType.bypass,
                        replica_groups=[list(range(NCORES))],
                        ins=[rev2[:].rearrange("k p t -> (k p) t")],
                        outs=[ag2[:]])
                    with tc.tile_pool(name="pE", bufs=1) as pE, \
                         tc.tile_pool(name="psE", bufs=4, space="PSUM") as psE:
                        xp2 = [pE.tile([128, T * BQ], BF16, name=f"xp2_{k}")
                               for k in range(4)]
                        for k in range(4):
                            nc.gpsimd.indirect_dma_start(
                                out=xp2[k][:], out_offset=None, in_=ag2[:],
                                in_offset=bass.IndirectOffsetOnAxis(
                                    ap=idxh[:, k:k + 1], axis=0))
                        fcw = [pE.tile([128, 64], BF16, name=f"fcwsb{k}")
                               for k in range(8)]
                        for k in range(8):
                            nc.sync.dma_start(fcw[k][:], fcw_d[k])
                        for m in range(G):
                            pg = psE.tile([128, 64], F32, tag="pgE")
                            nc.tensor.matmul(pg[:], lhsT=onesb[:], rhs=fcb[:],
                                             start=True, stop=False)
                            for k in range(8):
                                st = (hist2[k] if k < 4 else xp2[k - 4])
                                nc.tensor.matmul(
                                    pg[:], lhsT=st[:, 128 * m:128 * (m + 1)],
                                    rhs=fcw[k][:, :],
                                    start=False, stop=(k == 7))
                            lst = pE.tile([128, 64], F32, tag="lst", bufs=3)
                            nc.vector.tensor_copy(lst[:], pg[:])
                            nc.sync.dma_start(
                                logits_d[128 * m:128 * (m + 1), :], lst[:])

    nc.compile()
    return nc


# ---------------- host-side data prep ----------------

GPERM = [0, 1, 3, 2]  # gate order in device layout: i, f, o, g


def _reorder_cols(w):
    # w: [2048(gates i,f,g,o), D] -> moving layout [D, 2048] with
    # col(512j + 128g' + q) = w[512*GPERM[g'] + 128j + q, :]
    D = w.shape[1]
    w5 = w.reshape(4, 4, 128, D)[GPERM]        # [g', j, q, e]
    return np.transpose(w5, (3, 1, 0, 2)).reshape(D, 2048)


def _reorder_bias(b):
    b5 = b.reshape(4, 4, 128)[GPERM]           # [g', j, q]
    return np.transpose(b5, (1, 0, 2)).reshape(2048)


def _wmov(whh):
    # whh: [2048, 512] -> [128, 8192]: Wmov[p, (k*4+j)*512 + 128g' + q]
    w5 = whh.reshape(4, 4, 128, 4, 128)[GPERM]  # [g', j, q, k, p]
    return np.transpose(w5, (4, 3, 1, 0, 2)).reshape(128, 8192)


def _wih_mov(wih):
    # wih: [2048, D] -> [D//128, 128, 2048] K-chunk tiles of reordered cols
    D = wih.shape[1]
    r = _reorder_cols(wih)                     # [D, 2048]
    return r.reshape(D // 128, 128, 2048)


_CACHE = {}


def kernel(x, lengths, emb,
           Wih_f1, Whh_f1, bih_f1, bhh_f1,
           Wih_b1, Whh_b1, bih_b1, bhh_b1,
           Wih_f2, Whh_f2, bih_f2, bhh_f2,
           Wih_b2, Whh_b2, bih_b2, bhh_b2,
           fc_W, fc_b, _T=None):
    x = np.asarray(x)
    T = x.shape[1] if _T is None else _T
    G = T * BQ // 128

    if T not in _CACHE:
        _CACHE[T] = _build(T)
    nc = _CACHE[T]

    emb = np.asarray(emb, np.float32)
    f32 = lambda a: np.asarray(a, np.float32)
    layers = {
        0: (f32(Wih_f1), f32(Whh_f1), f32(bih_f1) + f32(bhh_f1),
            f32(Wih_f2), f32(Whh_f2), f32(bih_f2) + f32(bhh_f2)),
        1: (f32(Wih_b1), f32(Whh_b1), f32(bih_b1) + f32(bhh_b1),
            f32(Wih_b2), f32(Whh_b2), f32(bih_b2) + f32(bhh_b2)),
    }
    fc_W = f32(fc_W)
    fc_b = f32(fc_b)

    i16_np = np.zeros((16, 32), BF16_NP)
    i16_np[0:8, 0:8] = np.eye(8)
    i16_np[8:16, 0:8] = np.eye(8)
    common = {
        "emb": emb,
        "i16": i16_np,
        "onesr": np.ones((1, 128), np.float32),
        "onesb": np.ones((1, 128), BF16_NP),
        "id128": np.eye(128, dtype=np.float32),
        "fcb": np.pad(fc_b, (0, 14)).reshape(1, 64).astype(BF16_NP),
    }

    in_maps = []
    for i in range(NCORES):
        d, q = i // 4, i % 4
        wih1, whh1, bsum1, wih2, whh2, bsum2 = layers[d]
        xq = np.asarray(x[BQ * q:BQ * (q + 1), :T], np.int32)
        if d == 1:
            xq = xq[:, ::-1]
        # idx[p, g]: row r = 128g + p = 16t + b -> token xq[b, t]
        rr = np.arange(T * BQ)
        tt, bb = rr // BQ, rr % BQ
        idx_np = xq[bb, tt].reshape(G, 128).T.astype(np.int32).copy()
        peer = (i + 4) % 8
        pp = np.arange(128)
        idxh_np = (peer * 512 + 128 * np.arange(4)[None, :] + pp[:, None]
                   ).astype(np.int32)
        # layer-2 input feature order: own direction first, then peer
        own_sl = slice(512 * d, 512 * (d + 1))
        peer_sl = slice(512 * (1 - d), 512 * (2 - d))
        w2eff = np.concatenate([wih2[:, own_sl], wih2[:, peer_sl]], axis=1)
        fceff = np.concatenate([fc_W[:, own_sl], fc_W[:, peer_sl]], axis=1)
        fcmov = np.zeros((8, 128, 64), BF16_NP)
        for k in range(8):
            fcmov[k, :, :TAGS] = fceff[:, 128 * k:128 * (k + 1)].T
        in_maps.append(dict(
            common,
            idx=idx_np,
            idxh=idxh_np,
            w1=_wih_mov(wih1).astype(np.float32),
            b1=_reorder_bias(bsum1).reshape(1, 2048).astype(np.float32),
            wm1=_wmov(whh1).astype(BF16_NP),
            w2=_wih_mov(w2eff).astype(BF16_NP),
            b2=_reorder_bias(bsum2).reshape(1, 2048).astype(BF16_NP),
            wm2=_wmov(whh2).astype(BF16_NP),
            fcw=fcmov,
        ))

    res = run_bass_kernel_spmd(nc, in_maps, core_ids=list(range(NCORES)))

    out = np.zeros((B, T, TAGS), np.float32)
    for q in range(4):
        lg = res.results[q]["logits"][:, :TAGS]
        out[BQ * q:BQ * (q + 1)] = lg.reshape(T, BQ, TAGS).transpose(1, 0, 2)
    return out

